# revision 15
# baseline (speedup 1.0000x reference)
"""Trainium2 Bass kernel for nn_DiffusionLayer (gnn_message_passing).

Computation (full shapes, fp32 logical):
  x (16,64,64,512), A (16,512,64,64), phys_prior (16,64,512) ->
  corr (16,32,64,512)

Sharding: pure data parallel over batch B=16 across 8 cores (B_LOC=2 each).

v3 strategy:
  * fp16 on the wire (harness gate is 2e-2; measured ~3e-4..1e-3).
    Halves every HBM stream and runs PE matmuls at 1 cyc/row.
  * Host pre-packs x/A into the SBUF layouts the engines need, so every
    DMA is contiguous at line rate:
      x_pe[b, q, (f2 c), (fp mq)]    -- m-quarter-blocked stage-1 rhs
      a_pe[b, t, (m0 c), (m1 d)]     -- As-matmul lhsT, t = 32-m tile
      out  [b, h, og, c, o, mh]      -- 4KiB write runs, host transposes
  * m-quarter software pipeline on ONE input queue, x front-loaded:
      x0 x1 A0 x2 A1 x3 A2 A3
    so s(q) is ready just before A(q) arrives; deg/As drain each A tile
    on arrival; combine lags one quarter; PE stays continuously busy
    (stage-1 MMs of q+1 interleave with As MMs of q) to hold 2.4 GHz.
  * r-MLP needs mean_m over ALL m, so DT*r (rdt) is folded into the
    1x1-conv bias instead of the combine: out = snew'*pw + (rdt*pw+pb),
    letting combine(q) run per-quarter without waiting for full s.
  * deg reduce outputs fp16 (packed 2-byte operands -> DVE 2x mode);
    dedicated PSUM tiles per (quarter, b) so PE never waits on DVE.

Per-core traffic ~20.25 MiB -> ~57 us floor at 358 GB/s.
"""

import sys
import numpy as np

sys.path.insert(0, "/opt/trn_rl_repo")

import concourse.bass as bass  # noqa: E402
from concourse import bacc  # noqa: E402
import concourse.tile as tile  # noqa: E402
from concourse import mybir  # noqa: E402
from concourse.bass_utils import run_bass_kernel_spmd  # noqa: E402

B, F_DIM, C, M = 16, 64, 64, 512
OUT_CH = 32
DT = 1.0
N_CORES = 8
B_LOC = B // N_CORES  # 2
F32 = mybir.dt.float32
F16 = mybir.dt.float16
M_T = 32  # m's per A tile
NT = M // M_T  # 16 A tiles per b
NFP = F_DIM // 2  # 32 f-pairs
FPG = 8  # f-pairs per x DMA chunk
NQ = 4  # m-quarters
MBH = M // NQ  # 128 m's per quarter
TPQ = NT // NQ  # 4 A tiles per (b, quarter)
MH2 = M // 2  # 256 m's per out half
OG = 8  # out channels per DMA

_CACHE = {}


def _build_bass():
    nc = bacc.Bacc()

    x_sh = nc.declare_dram_parameter(
        "x_sh", [B_LOC, NQ, 128, NFP * MBH], F16, isOutput=False
    )
    a_sh = nc.declare_dram_parameter(
        "a_sh", [B_LOC, NT, 128, (M_T // 2) * C], F16, isOutput=False
    )
    pp_sh = nc.declare_dram_parameter("pp_sh", [B_LOC, C, M], F32, isOutput=False)
    ones_bd = nc.declare_dram_parameter("ones_bd", [128, C], F16, isOutput=False)
    w1r = nc.declare_dram_parameter("w1r", [128, 16], F32, isOutput=False)
    b1r = nc.declare_dram_parameter("b1r", [128, 16], F32, isOutput=False)
    w2r = nc.declare_dram_parameter("w2r", [128, 16], F32, isOutput=False)
    cvec = nc.declare_dram_parameter("cvec", [128, 4], F32, isOutput=False)
    pwpb = nc.declare_dram_parameter("pwpb", [128, 2 * OUT_CH], F32, isOutput=False)
    out_sh = nc.declare_dram_parameter(
        "out", [B_LOC, 2, OUT_CH // OG, C, OG, MH2], F16, isOutput=True
    )

    AX = mybir.AxisListType
    OP = mybir.AluOpType
    ACTF = mybir.ActivationFunctionType

    with tile.TileContext(nc) as tc:
        with (
            tc.tile_pool(name="const16", bufs=1) as cpool16,
            tc.tile_pool(name="const", bufs=1) as cpool,
            tc.tile_pool(name="xp", bufs=3) as xpool,
            tc.tile_pool(name="ap", bufs=3) as apool,
            tc.tile_pool(name="sp", bufs=1) as spool,
            tc.tile_pool(name="tmp", bufs=2) as tpool,
            tc.tile_pool(name="dpk", bufs=4) as dpkpool,
            tc.tile_pool(name="fold", bufs=3) as fpool,
            tc.tile_pool(name="small", bufs=1) as smpool,
            tc.tile_pool(name="op", bufs=2) as opool,
            tc.tile_pool(name="ps_s", bufs=1, space="PSUM") as ps_s_pool,
            tc.tile_pool(name="ps_as", bufs=1, space="PSUM") as ps_as_pool,
        ):
            # ---- constants on the (idle-at-start) scalar ring ----
            ones_t = cpool16.tile([128, C], F16)
            nc.scalar.dma_start(ones_t[:], ones_bd[:])
            NCC = 16 * 3 + 4 + 2 * OUT_CH
            call_t = cpool.tile([128, NCC], F32)
            nc.scalar.dma_start(call_t[:, 0:16], w1r[:])
            nc.scalar.dma_start(call_t[:, 16:32], b1r[:])
            nc.scalar.dma_start(call_t[:, 32:48], w2r[:])
            nc.scalar.dma_start(call_t[:, 48:52], cvec[:])
            nc.scalar.dma_start(call_t[:, 52:NCC], pwpb[:])
            w1r_t = call_t[:, 0:16]
            b1r_t = call_t[:, 16:32]
            w2r_t = call_t[:, 32:48]
            cvec_t = call_t[:, 48:52]
            pwpb_t = call_t[:, 52:NCC]
            pp_t = spool.tile([128, M], F32)
            nc.scalar.dma_start(pp_t[:], pp_sh[:])

            # persistent tiles
            s_ps = ps_s_pool.tile([128, M], F32)
            s_ps2 = ps_s_pool.tile([128, M], F32, name="s_ps2")
            s_t = spool.tile([128, M], F32)
            deg_t = spool.tile([128, M], F32)
            snew = spool.tile([128, M], F32)
            ppr = spool.tile([128, M], F32)  # DT*alpha*pp, precomputed
            rq = smpool.tile([128, NQ], F32)  # per-quarter sum_m s
            ones_f32 = smpool.tile([128, 1], F32)
            nc.vector.memset(ones_f32[:], 1.0)
            bo = smpool.tile([128, OUT_CH], F32)  # rdt*pw + pb out biases
            s_bds = [
                spool.tile([128, M], F16, name=f"sbd{b}") for b in range(B_LOC)
            ]
            otf = [
                spool.tile([128, OUT_CH * MH2], F16, name=f"otf{h}")
                for h in range(2)
            ]
            as_ps_b = [
                ps_as_pool.tile([128, M], F32, name=f"asps{b}")
                for b in range(B_LOC)
            ]

            def emit_x_quarter(q):
                qsl = slice(q * MBH, (q + 1) * MBH)
                for b in range(B_LOC):
                    xt = xpool.tile([128, NFP * MBH], F16)
                    nc.scalar.dma_start(xt[:], x_sh[b, q])
                    for fp in range(NFP):
                        # ping-pong two PSUM regions so consecutive
                        # accumulating MMs never RAW-serialize on the
                        # array drain
                        ps = s_ps if fp % 2 == 0 else s_ps2
                        nc.tensor.matmul(
                            ps[b * C : (b + 1) * C, qsl],
                            ones_t[:],
                            xt[:, fp * MBH : (fp + 1) * MBH],
                            start=(fp < 2),
                            stop=(fp >= NFP - 2),
                        )
                    bsl = slice(b * C, (b + 1) * C)
                    # s = psA + psB (the 1/64 mean is folded into ones_bd);
                    # DVE can read only one PSUM operand -> ACT evacuates psB
                    stmp = tpool.tile([128, MBH], F32, tag="stmp", name=f"stmp{b}_{q}")
                    nc.scalar.activation(
                        stmp[bsl, :], s_ps2[bsl, qsl], ACTF.Copy
                    )
                    nc.vector.tensor_add(s_t[bsl, qsl], s_ps[bsl, qsl], stmp[bsl, :])
                    bb = s_bds[b]
                    if q == 0:
                        nc.vector.memset(bb[:], 0.0)
                    nc.vector.tensor_copy(
                        bb[0:64, q * MBH : (q + 1) * MBH : 2],
                        s_t[bsl, q * MBH : (q + 1) * MBH : 2],
                    )
                    nc.vector.tensor_copy(
                        bb[64:128, q * MBH + 1 : (q + 1) * MBH : 2],
                        s_t[bsl, q * MBH + 1 : (q + 1) * MBH : 2],
                    )
                # per-quarter sum_m s for the r-MLP input
                nc.vector.tensor_reduce(
                    rq[:, q : q + 1], s_t[:, qsl], axis=AX.X, op=OP.add
                )

            MH = M_T // 2  # m1's per tile

            def emit_a_quarter(q):
                dpkq = [
                    dpkpool.tile(
                        [128, MBH // 2], F16, tag=f"dpk{b}", name=f"dpk{b}_{q}"
                    )
                    for b in range(B_LOC)
                ]
                at4 = {}
                for b in range(B_LOC):
                    at4[b] = apool.tile(
                        [128, TPQ * MH * C], F16, tag=f"at{b}", name=f"at{b}_{q}"
                    )
                    nc.sync.dma_start(
                        at4[b][:].rearrange("p (t f) -> p t f", t=TPQ),
                        a_sh[b, q * TPQ : (q + 1) * TPQ].rearrange(
                            "t p f -> p t f"
                        ),
                    )
                for tq in range(TPQ):
                    mt = q * TPQ + tq
                    for b in range(B_LOC):
                        at = at4[b][:, tq * MH * C : (tq + 1) * MH * C]
                        # deg: packed fp16 tree-fold d 64->32->16, then reduce
                        f1 = fpool.tile([128, MH * 32], F16, tag="f1")
                        av = at.rearrange("p (mm h d) -> p mm h d", h=2, d=32)
                        with nc.allow_low_precision(reason="deg fp16 tree"):
                            nc.vector.tensor_add(
                                f1[:].rearrange("p (mm d) -> p mm d", d=32),
                                av[:, :, 0, :],
                                av[:, :, 1, :],
                            )
                            f2 = fpool.tile([128, MH * 16], F16, tag="f2")
                            f1v = f1[:].rearrange(
                                "p (mm h d) -> p mm h d", h=2, d=16
                            )
                            nc.vector.tensor_add(
                                f2[:].rearrange("p (mm d) -> p mm d", d=16),
                                f1v[:, :, 0, :],
                                f1v[:, :, 1, :],
                            )
                            nc.vector.tensor_reduce(
                                dpkq[b][:, tq * MH : (tq + 1) * MH],
                                f2[:].rearrange("p (mm d) -> p mm d", d=16),
                                axis=AX.X,
                                op=OP.add,
                            )
                        for j in range(MH // 2):
                            me4 = mt * M_T + 4 * j
                            nc.tensor.matmul(
                                as_ps_b[b][:, me4 : me4 + 4],
                                at[:, 2 * j * C : (2 * j + 2) * C],
                                s_bds[b][:, me4 : me4 + 4],
                                start=True,
                                stop=True,
                            )

                # parity de-interleave: deg_t[c, m] (fp32) from dpkq[(m0,c), (tq,m1)]
                for b in range(B_LOC):
                    bsl = slice(b * C, (b + 1) * C)
                    nc.vector.tensor_copy(
                        deg_t[bsl, q * MBH : (q + 1) * MBH : 2], dpkq[b][0:64, :]
                    )
                    nc.vector.tensor_copy(
                        deg_t[bsl, q * MBH + 1 : (q + 1) * MBH : 2],
                        dpkq[b][64:128, :],
                    )

            def emit_combine(q):
                hs = slice(q * MBH, (q + 1) * MBH)
                t2p = tpool.tile([128, MBH], F32, tag="t2p")
                nc.scalar.activation(
                    t2p[:], deg_t[:, hs], ACTF.Identity,
                    scale=cvec_t[:, 0:1], bias=ones_f32[:],
                )
                t2 = tpool.tile([128, MBH], F32, tag="t2")
                nc.vector.tensor_mul(t2[:], t2p[:], s_t[:, hs])
                # t3 = DT*k*As: psum rows (m1-parity, d); valid half by
                # (m//2)%2: cols {4u,4u+1} -> rows 0:64, {4u+2,4u+3} -> 64:128
                t3 = tpool.tile([128, MBH], F32, tag="t3")
                kap = cvec_t[0:64, 1:2]
                for b in range(B_LOC):
                    bsl = slice(b * C, (b + 1) * C)
                    aps = as_ps_b[b][:, hs]
                    t3v = t3[bsl, :].rearrange("p (u k) -> p u k", k=4)
                    apse = aps[0:64, :].rearrange("p (u k) -> p u k", k=4)
                    apso = aps[64:128, :].rearrange("p (u k) -> p u k", k=4)
                    nc.scalar.activation(
                        t3v[:, :, 0:2], apse[:, :, 0:2], ACTF.Identity, scale=kap
                    )
                    nc.scalar.activation(
                        t3v[:, :, 2:4], apso[:, :, 2:4], ACTF.Identity, scale=kap
                    )
                t4 = tpool.tile([128, MBH], F32, tag="t4")
                nc.vector.tensor_add(t4[:], t2[:], t3[:])
                nc.vector.tensor_add(snew[:, hs], t4[:], ppr[:, hs])

            def emit_mlp_bo():
                # r-MLP on rin = mean_m s; rdt folded into out biases bo
                rsum = smpool.tile([128, 1], F32)
                nc.vector.tensor_reduce(rsum[:], rq[:], axis=AX.X, op=OP.add)
                rin = smpool.tile([128, 1], F32)
                nc.vector.tensor_scalar_mul(rin[:], rsum[:], 1.0 / M)
                hp = smpool.tile([128, 16], F32)
                nc.vector.tensor_scalar(hp[:], w1r_t[:], rin[:], None, op0=OP.mult)
                nc.vector.tensor_add(hp[:], hp[:], b1r_t[:])
                hneg = smpool.tile([128, 16], F32)
                nc.vector.tensor_scalar_min(hneg[:], hp[:], 0.0)
                hexp = smpool.tile([128, 16], F32)
                nc.scalar.activation(hexp[:], hneg[:], ACTF.Exp)
                hrelu = smpool.tile([128, 16], F32)
                nc.vector.tensor_scalar_max(hrelu[:], hp[:], 0.0)
                helu = smpool.tile([128, 16], F32)
                nc.vector.tensor_add(helu[:], hexp[:], hrelu[:])
                # helu = elu + 1; the -1 is folded into cvec[:,3] on host
                hw = smpool.tile([128, 16], F32)
                nc.vector.tensor_mul(hw[:], helu[:], w2r_t[:])
                rpre = smpool.tile([128, 1], F32)
                nc.vector.tensor_reduce(rpre[:], hw[:], axis=AX.X, op=OP.add)
                rdt = smpool.tile([128, 1], F32)
                nc.vector.tensor_scalar(
                    rdt[:], rpre[:], cvec_t[:, 3:4], None, op0=OP.add
                )
                # bo[:, o] = rdt*pw[o] + pb[o]
                nc.vector.tensor_scalar(
                    bo[:], pwpb_t[:, 0::2], rdt[:], None, op0=OP.mult
                )
                nc.vector.tensor_add(bo[:], bo[:], pwpb_t[:, 1::2])

            def emit_out_h0():
                # all 32 channels for m-half 0 (quarters 0&1), ACT-heavy
                # since DVE is busy with deg; DMAs ride the scalar ring
                hs = slice(0, MH2)
                for og in range(OUT_CH // OG):
                    for g in range(OG):
                        o = og * OG + g
                        dst = otf[0][:, o * MH2 : (o + 1) * MH2]
                        if g < 3:
                            nc.vector.tensor_scalar(
                                dst, snew[:, hs],
                                pwpb_t[:, 2 * o : 2 * o + 1],
                                bo[:, o : o + 1], op0=OP.mult, op1=OP.add,
                            )
                        else:
                            nc.scalar.activation(
                                dst, snew[:, hs], ACTF.Identity,
                                bias=bo[:, o : o + 1],
                                scale=pwpb_t[:, 2 * o : 2 * o + 1],
                            )
                if og == OUT_CH // OG - 1:
                    for b in range(B_LOC):
                        osrc = otf[0][b * C : (b + 1) * C, :].rearrange(
                            "p (og o m) -> p og o m", og=4, m=MH2
                        )
                        nc.scalar.dma_start(
                            out_sh[b, 0].rearrange("og c o m -> c og o m"), osrc
                        )

            def emit_out_h1_part(qp):
                # quarter-slice (q = 2 + qp) of half 1 for all 32 channels
                qs2 = slice((2 + qp) * MBH, (3 + qp) * MBH)
                for og in range(OUT_CH // OG):
                    for g in range(OG):
                        o = og * OG + g
                        dst = otf[1][
                            :, o * MH2 + qp * MBH : o * MH2 + (qp + 1) * MBH
                        ]
                        if g < 5:
                            nc.vector.tensor_scalar(
                                dst, snew[:, qs2],
                                pwpb_t[:, 2 * o : 2 * o + 1],
                                bo[:, o : o + 1], op0=OP.mult, op1=OP.add,
                            )
                        else:
                            nc.scalar.activation(
                                dst, snew[:, qs2], ACTF.Identity,
                                bias=bo[:, o : o + 1],
                                scale=pwpb_t[:, 2 * o : 2 * o + 1],
                            )
                    if qp == 1 and og == OUT_CH // OG - 1:
                        # all channels done -> big DMAs on the (now idle)
                        # sync ring
                        for b in range(B_LOC):
                            osrc = otf[1][b * C : (b + 1) * C, :].rearrange(
                                "p (og o m) -> p og o m", og=4, m=MH2
                            )
                            nc.sync.dma_start(
                                out_sh[b, 1].rearrange("og c o m -> c og o m"),
                                osrc,
                            )

            # ---- schedule: x front-loaded, combine lags one quarter ----
            emit_x_quarter(0)
            nc.vector.tensor_scalar(
                ppr[:], pp_t[:], cvec_t[:, 2:3], None, op0=OP.mult
            )
            emit_x_quarter(1)
            emit_a_quarter(0)
            emit_x_quarter(2)
            emit_combine(0)
            emit_a_quarter(1)
            emit_x_quarter(3)
            emit_mlp_bo()
            emit_combine(1)
            emit_out_h0()
            emit_a_quarter(2)
            emit_combine(2)
            emit_out_h1_part(0)
            emit_a_quarter(3)
            emit_combine(3)
            emit_out_h1_part(1)

    nc.compile()
    return nc


def _get_bass():
    if "nc" not in _CACHE:
        _CACHE["nc"] = _build_bass()
    return _CACHE["nc"]


def _host_consts(kappa, alpha, w1, b1, w2, b2, pw, pb):
    kappa = float(np.asarray(kappa))
    alpha = float(np.asarray(alpha))
    w1 = np.asarray(w1, np.float32).reshape(16, 1)
    b1 = np.asarray(b1, np.float32).reshape(16)
    w2 = np.asarray(w2, np.float32).reshape(1, 16)
    b2 = np.asarray(b2, np.float32).reshape(1)
    pw = np.asarray(pw, np.float32).reshape(OUT_CH)
    pb = np.asarray(pb, np.float32).reshape(OUT_CH)

    kDT = DT * float(np.log1p(np.exp(kappa)))  # DT * softplus(kappa)

    # blockdiag "ones" carry the 1/F_DIM mean scale (exact in fp16)
    ones_bd = np.zeros((128, C), np.float16)
    for f in range(2):
        for c in range(C):
            ones_bd[f * C + c, c] = 1.0 / F_DIM

    w1r = np.tile(w1.T.astype(np.float32), (128, 1))  # [128,16]
    b1r = np.tile(b1[None, :], (128, 1)).astype(np.float32)
    w2r_dt = np.tile((DT * w2).astype(np.float32), (128, 1))  # [128,16]

    cvec = np.zeros((128, 4), np.float32)
    cvec[:, 0] = -kDT
    cvec[:, 1] = kDT
    cvec[:, 2] = DT * alpha
    # rdt = rpre + cvec3 where rpre = sum(w2r_dt * (elu+1));
    # true DT*r = sum(w2r_dt*elu) + DT*b2  ->  cvec3 = DT*b2 - sum(w2r_dt row)
    cvec[:, 3] = DT * b2[0] - float(w2r_dt[0].sum())

    pwpb = np.zeros((128, 2 * OUT_CH), np.float32)
    pwpb[:, 0::2] = pw[None, :]
    pwpb[:, 1::2] = pb[None, :]
    return ones_bd, w1r, b1r, w2r_dt, cvec, pwpb


def _host_prep(x, A, phys_prior, kappa, alpha, w1, b1, w2, b2, pw, pb):
    """Pack full inputs into per-core in_maps (fp16 blocked layouts)."""
    x = np.asarray(x, np.float32)
    A = np.asarray(A, np.float32)
    pp = np.ascontiguousarray(np.asarray(phys_prior, np.float32))

    # x_pe[b, q, f2*64+c, fp*128+mq] = x[b, 2*fp+f2, c, q*128+mq]
    x16 = x.astype(np.float16).reshape(B, NFP, 2, C, NQ, MBH)
    x_pe = np.ascontiguousarray(x16.transpose(0, 4, 2, 3, 1, 5)).reshape(
        B, NQ, 128, NFP * MBH
    )
    # a_pe[b, t, m0*64+c, m1*64+d] = A[b, t*32+2*m1+m0, c, d]
    A16 = A.astype(np.float16).reshape(B, NT, M_T // 2, 2, C, C)
    a_pe = np.ascontiguousarray(A16.transpose(0, 1, 3, 4, 2, 5)).reshape(
        B, NT, 128, (M_T // 2) * C
    )

    ones_bd, w1r, b1r, w2r_dt, cvec, pwpb = _host_consts(
        kappa, alpha, w1, b1, w2, b2, pw, pb
    )

    in_maps = []
    for i in range(N_CORES):
        sl = slice(i * B_LOC, (i + 1) * B_LOC)
        in_maps.append(
            {
                "x_sh": x_pe[sl],
                "a_sh": a_pe[sl],
                "pp_sh": pp[sl],
                "ones_bd": ones_bd,
                "w1r": w1r,
                "b1r": b1r,
                "w2r": w2r_dt,
                "cvec": cvec,
                "pwpb": pwpb,
            }
        )
    return in_maps


def _host_post(res):
    """Gather per-core fp16 [B_LOC,2,4,C,OG,MH2] outputs -> fp32 (B,O,C,M)."""
    out16 = np.concatenate(
        [res.results[i]["out"] for i in range(N_CORES)], axis=0
    )  # (B, 2, 4, C, OG, MH2)
    # o = og*OG + g, m = h*MH2 + mh
    out = out16.transpose(0, 2, 4, 3, 1, 5).reshape(B, OUT_CH, C, M)
    return np.ascontiguousarray(out).astype(np.float32)


def kernel(x, A, phys_prior, kappa, alpha, w1, b1, w2, b2, pw, pb):
    in_maps = _host_prep(
        x, A, phys_prior, kappa, alpha, w1, b1, w2, b2, pw, pb
    )
    nc = _get_bass()
    res = run_bass_kernel_spmd(nc, in_maps, list(range(N_CORES)))
    return _host_post(res)


if __name__ == "__main__":
    # smoke test with random data
    rng = np.random.default_rng(0)
    inputs = dict(
        x=rng.standard_normal((B, F_DIM, C, M)).astype(np.float32),
        A=rng.random((B, M, C, C)).astype(np.float32),
        phys_prior=rng.standard_normal((B, C, M)).astype(np.float32),
        kappa=np.float32(0.1),
        alpha=np.float32(0.05),
        w1=rng.standard_normal((16, 1)).astype(np.float32),
        b1=np.zeros(16, np.float32),
        w2=(rng.standard_normal((1, 16)) * 0.25).astype(np.float32),
        b2=np.zeros(1, np.float32),
        pw=rng.standard_normal(OUT_CH).astype(np.float32),
        pb=np.zeros(OUT_CH, np.float32),
    )
    out = kernel(**inputs)
    print("out", out.shape, out.dtype)


# revision 16
# speedup vs baseline: 1.0767x; 1.0767x over previous
"""Trainium2 Bass kernel for nn_DiffusionLayer (gnn_message_passing).

Computation (full shapes, fp32 logical):
  x (16,64,64,512), A (16,512,64,64), phys_prior (16,64,512) ->
  corr (16,32,64,512)

Sharding: pure data parallel over batch B=16 across 8 cores (B_LOC=2 each).

v3 strategy:
  * fp16 on the wire (harness gate is 2e-2; measured ~3e-4..1e-3).
    Halves every HBM stream and runs PE matmuls at 1 cyc/row.
  * Host pre-packs x/A into the SBUF layouts the engines need, so every
    DMA is contiguous at line rate:
      x_pe[b, q, (f2 c), (fp mq)]    -- m-quarter-blocked stage-1 rhs
      a_pe[b, t, (m0 c), (m1 d)]     -- As-matmul lhsT, t = 32-m tile
      out  [b, h, og, c, o, mh]      -- 4KiB write runs, host transposes
  * m-quarter software pipeline on ONE input queue, x front-loaded:
      x0 x1 A0 x2 A1 x3 A2 A3
    so s(q) is ready just before A(q) arrives; deg/As drain each A tile
    on arrival; combine lags one quarter; PE stays continuously busy
    (stage-1 MMs of q+1 interleave with As MMs of q) to hold 2.4 GHz.
  * r-MLP needs mean_m over ALL m, so DT*r (rdt) is folded into the
    1x1-conv bias instead of the combine: out = snew'*pw + (rdt*pw+pb),
    letting combine(q) run per-quarter without waiting for full s.
  * deg reduce outputs fp16 (packed 2-byte operands -> DVE 2x mode);
    dedicated PSUM tiles per (quarter, b) so PE never waits on DVE.

Per-core traffic ~20.25 MiB -> ~57 us floor at 358 GB/s.
"""

import sys
import numpy as np

sys.path.insert(0, "/opt/trn_rl_repo")

import concourse.bass as bass  # noqa: E402
from concourse import bacc  # noqa: E402
import concourse.tile as tile  # noqa: E402
from concourse import mybir  # noqa: E402
from concourse.bass_utils import run_bass_kernel_spmd  # noqa: E402

B, F_DIM, C, M = 16, 64, 64, 512
OUT_CH = 32
DT = 1.0
N_CORES = 8
B_LOC = B // N_CORES  # 2
F32 = mybir.dt.float32
F16 = mybir.dt.float16
M_T = 32  # m's per A tile
NT = M // M_T  # 16 A tiles per b
NFP = F_DIM // 2  # 32 f-pairs
FPG = 8  # f-pairs per x DMA chunk
NQ = 4  # m-quarters
MBH = M // NQ  # 128 m's per quarter
TPQ = NT // NQ  # 4 A tiles per (b, quarter)
MH2 = M // 2  # 256 m's per out half
OG = 8  # out channels per DMA

_CACHE = {}


def _build_bass():
    nc = bacc.Bacc()

    x_sh = nc.declare_dram_parameter(
        "x_sh", [B_LOC, NQ, 128, NFP * MBH], F16, isOutput=False
    )
    a_sh = nc.declare_dram_parameter(
        "a_sh", [B_LOC, NT, 128, (M_T // 2) * C], F16, isOutput=False
    )
    pp_sh = nc.declare_dram_parameter("pp_sh", [B_LOC, C, M], F32, isOutput=False)
    ones_bd = nc.declare_dram_parameter("ones_bd", [128, C], F16, isOutput=False)
    w1r = nc.declare_dram_parameter("w1r", [128, 16], F32, isOutput=False)
    b1r = nc.declare_dram_parameter("b1r", [128, 16], F32, isOutput=False)
    w2r = nc.declare_dram_parameter("w2r", [128, 16], F32, isOutput=False)
    cvec = nc.declare_dram_parameter("cvec", [128, 4], F32, isOutput=False)
    pwpb = nc.declare_dram_parameter("pwpb", [128, 2 * OUT_CH], F32, isOutput=False)
    out_sh = nc.declare_dram_parameter(
        "out", [B_LOC, 2, OUT_CH // OG, C, OG, MH2], F16, isOutput=True
    )

    AX = mybir.AxisListType
    OP = mybir.AluOpType
    ACTF = mybir.ActivationFunctionType

    with tile.TileContext(nc) as tc:
        with (
            tc.tile_pool(name="const16", bufs=1) as cpool16,
            tc.tile_pool(name="const", bufs=1) as cpool,
            tc.tile_pool(name="xp", bufs=3) as xpool,
            tc.tile_pool(name="ap", bufs=3) as apool,
            tc.tile_pool(name="sp", bufs=1) as spool,
            tc.tile_pool(name="tmp", bufs=2) as tpool,
            tc.tile_pool(name="dpk", bufs=4) as dpkpool,
            tc.tile_pool(name="fold", bufs=3) as fpool,
            tc.tile_pool(name="small", bufs=1) as smpool,
            tc.tile_pool(name="op", bufs=2) as opool,
            tc.tile_pool(name="ps_s", bufs=1, space="PSUM") as ps_s_pool,
            tc.tile_pool(name="ps_as", bufs=1, space="PSUM") as ps_as_pool,
        ):
            # ---- constants on the (idle-at-start) scalar ring ----
            ones_t = cpool16.tile([128, C], F16)
            nc.scalar.dma_start(ones_t[:], ones_bd[:])
            NCC = 16 * 3 + 4 + 2 * OUT_CH
            call_t = cpool.tile([128, NCC], F32)
            nc.scalar.dma_start(call_t[:, 0:16], w1r[:])
            nc.scalar.dma_start(call_t[:, 16:32], b1r[:])
            nc.scalar.dma_start(call_t[:, 32:48], w2r[:])
            nc.scalar.dma_start(call_t[:, 48:52], cvec[:])
            nc.scalar.dma_start(call_t[:, 52:NCC], pwpb[:])
            w1r_t = call_t[:, 0:16]
            b1r_t = call_t[:, 16:32]
            w2r_t = call_t[:, 32:48]
            cvec_t = call_t[:, 48:52]
            pwpb_t = call_t[:, 52:NCC]
            pp_t = spool.tile([128, M], F32)
            nc.scalar.dma_start(pp_t[:], pp_sh[:])

            # persistent tiles
            s_ps = ps_s_pool.tile([128, M], F32)
            s_ps2 = ps_s_pool.tile([128, M], F32, name="s_ps2")
            s_t = spool.tile([128, M], F32)
            deg_t = spool.tile([128, M], F32)
            snew = spool.tile([128, M], F32)
            ppr = spool.tile([128, M], F32)  # DT*alpha*pp, precomputed
            rq = smpool.tile([128, NQ], F32)  # per-quarter sum_m s
            ones_f32 = smpool.tile([128, 1], F32)
            nc.vector.memset(ones_f32[:], 1.0)
            bo = smpool.tile([128, OUT_CH], F32)  # rdt*pw + pb out biases
            s_bds = [
                spool.tile([128, M], F16, name=f"sbd{b}") for b in range(B_LOC)
            ]
            otf = [
                spool.tile([128, OUT_CH * MH2], F16, name=f"otf{h}")
                for h in range(2)
            ]
            as_ps_b = [
                ps_as_pool.tile([128, M], F32, name=f"asps{b}")
                for b in range(B_LOC)
            ]

            def emit_x_quarter(q):
                qsl = slice(q * MBH, (q + 1) * MBH)
                for b in range(B_LOC):
                    xt = xpool.tile([128, NFP * MBH], F16)
                    nc.sync.dma_start(xt[:], x_sh[b, q])
                    for fp in range(NFP):
                        # ping-pong two PSUM regions so consecutive
                        # accumulating MMs never RAW-serialize on the
                        # array drain
                        ps = s_ps if fp % 2 == 0 else s_ps2
                        nc.tensor.matmul(
                            ps[b * C : (b + 1) * C, qsl],
                            ones_t[:],
                            xt[:, fp * MBH : (fp + 1) * MBH],
                            start=(fp < 2),
                            stop=(fp >= NFP - 2),
                        )
                    bsl = slice(b * C, (b + 1) * C)
                    # s = psA + psB (the 1/64 mean is folded into ones_bd);
                    # DVE can read only one PSUM operand -> ACT evacuates psB
                    stmp = tpool.tile([128, MBH], F32, tag="stmp", name=f"stmp{b}_{q}")
                    nc.scalar.activation(
                        stmp[bsl, :], s_ps2[bsl, qsl], ACTF.Copy
                    )
                    nc.vector.tensor_add(s_t[bsl, qsl], s_ps[bsl, qsl], stmp[bsl, :])
                    bb = s_bds[b]
                    if q == 0:
                        nc.vector.memset(bb[:], 0.0)
                    nc.vector.tensor_copy(
                        bb[0:64, q * MBH : (q + 1) * MBH : 2],
                        s_t[bsl, q * MBH : (q + 1) * MBH : 2],
                    )
                    nc.vector.tensor_copy(
                        bb[64:128, q * MBH + 1 : (q + 1) * MBH : 2],
                        s_t[bsl, q * MBH + 1 : (q + 1) * MBH : 2],
                    )
                # per-quarter sum_m s for the r-MLP input
                nc.vector.tensor_reduce(
                    rq[:, q : q + 1], s_t[:, qsl], axis=AX.X, op=OP.add
                )

            MH = M_T // 2  # m1's per tile

            def emit_a_quarter(q):
                dpkq = [
                    dpkpool.tile(
                        [128, MBH // 2], F16, tag=f"dpk{b}", name=f"dpk{b}_{q}"
                    )
                    for b in range(B_LOC)
                ]
                at4 = {}
                for b in range(B_LOC):
                    at4[b] = apool.tile(
                        [128, TPQ * MH * C], F16, tag=f"at{b}", name=f"at{b}_{q}"
                    )
                    nc.sync.dma_start(
                        at4[b][:].rearrange("p (t f) -> p t f", t=TPQ),
                        a_sh[b, q * TPQ : (q + 1) * TPQ].rearrange(
                            "t p f -> p t f"
                        ),
                    )
                for tq in range(TPQ):
                    mt = q * TPQ + tq
                    for b in range(B_LOC):
                        at = at4[b][:, tq * MH * C : (tq + 1) * MH * C]
                        # deg: packed fp16 tree-fold d 64->32->16, then reduce
                        f1 = fpool.tile([128, MH * 32], F16, tag="f1")
                        av = at.rearrange("p (mm h d) -> p mm h d", h=2, d=32)
                        with nc.allow_low_precision(reason="deg fp16 tree"):
                            nc.vector.tensor_add(
                                f1[:].rearrange("p (mm d) -> p mm d", d=32),
                                av[:, :, 0, :],
                                av[:, :, 1, :],
                            )
                            f2 = fpool.tile([128, MH * 16], F16, tag="f2")
                            f1v = f1[:].rearrange(
                                "p (mm h d) -> p mm h d", h=2, d=16
                            )
                            nc.vector.tensor_add(
                                f2[:].rearrange("p (mm d) -> p mm d", d=16),
                                f1v[:, :, 0, :],
                                f1v[:, :, 1, :],
                            )
                            nc.vector.tensor_reduce(
                                dpkq[b][:, tq * MH : (tq + 1) * MH],
                                f2[:].rearrange("p (mm d) -> p mm d", d=16),
                                axis=AX.X,
                                op=OP.add,
                            )
                        for j in range(MH // 2):
                            me4 = mt * M_T + 4 * j
                            nc.tensor.matmul(
                                as_ps_b[b][:, me4 : me4 + 4],
                                at[:, 2 * j * C : (2 * j + 2) * C],
                                s_bds[b][:, me4 : me4 + 4],
                                start=True,
                                stop=True,
                            )

                # parity de-interleave: deg_t[c, m] (fp32) from dpkq[(m0,c), (tq,m1)]
                for b in range(B_LOC):
                    bsl = slice(b * C, (b + 1) * C)
                    nc.vector.tensor_copy(
                        deg_t[bsl, q * MBH : (q + 1) * MBH : 2], dpkq[b][0:64, :]
                    )
                    nc.vector.tensor_copy(
                        deg_t[bsl, q * MBH + 1 : (q + 1) * MBH : 2],
                        dpkq[b][64:128, :],
                    )

            def emit_combine(q):
                hs = slice(q * MBH, (q + 1) * MBH)
                t2p = tpool.tile([128, MBH], F32, tag="t2p")
                nc.scalar.activation(
                    t2p[:], deg_t[:, hs], ACTF.Identity,
                    scale=cvec_t[:, 0:1], bias=ones_f32[:],
                )
                t2 = tpool.tile([128, MBH], F32, tag="t2")
                nc.vector.tensor_mul(t2[:], t2p[:], s_t[:, hs])
                # t3 = DT*k*As: psum rows (m1-parity, d); valid half by
                # (m//2)%2: cols {4u,4u+1} -> rows 0:64, {4u+2,4u+3} -> 64:128
                t3 = tpool.tile([128, MBH], F32, tag="t3")
                kap = cvec_t[0:64, 1:2]
                for b in range(B_LOC):
                    bsl = slice(b * C, (b + 1) * C)
                    aps = as_ps_b[b][:, hs]
                    t3v = t3[bsl, :].rearrange("p (u k) -> p u k", k=4)
                    apse = aps[0:64, :].rearrange("p (u k) -> p u k", k=4)
                    apso = aps[64:128, :].rearrange("p (u k) -> p u k", k=4)
                    nc.scalar.activation(
                        t3v[:, :, 0:2], apse[:, :, 0:2], ACTF.Identity, scale=kap
                    )
                    nc.scalar.activation(
                        t3v[:, :, 2:4], apso[:, :, 2:4], ACTF.Identity, scale=kap
                    )
                t4 = tpool.tile([128, MBH], F32, tag="t4")
                nc.vector.tensor_add(t4[:], t2[:], t3[:])
                nc.vector.tensor_add(snew[:, hs], t4[:], ppr[:, hs])

            def emit_mlp_bo():
                # r-MLP on rin = mean_m s; rdt folded into out biases bo
                rsum = smpool.tile([128, 1], F32)
                nc.vector.tensor_reduce(rsum[:], rq[:], axis=AX.X, op=OP.add)
                rin = smpool.tile([128, 1], F32)
                nc.vector.tensor_scalar_mul(rin[:], rsum[:], 1.0 / M)
                hp = smpool.tile([128, 16], F32)
                nc.vector.tensor_scalar(hp[:], w1r_t[:], rin[:], None, op0=OP.mult)
                nc.vector.tensor_add(hp[:], hp[:], b1r_t[:])
                hneg = smpool.tile([128, 16], F32)
                nc.vector.tensor_scalar_min(hneg[:], hp[:], 0.0)
                hexp = smpool.tile([128, 16], F32)
                nc.scalar.activation(hexp[:], hneg[:], ACTF.Exp)
                hrelu = smpool.tile([128, 16], F32)
                nc.vector.tensor_scalar_max(hrelu[:], hp[:], 0.0)
                helu = smpool.tile([128, 16], F32)
                nc.vector.tensor_add(helu[:], hexp[:], hrelu[:])
                # helu = elu + 1; the -1 is folded into cvec[:,3] on host
                hw = smpool.tile([128, 16], F32)
                nc.vector.tensor_mul(hw[:], helu[:], w2r_t[:])
                rpre = smpool.tile([128, 1], F32)
                nc.vector.tensor_reduce(rpre[:], hw[:], axis=AX.X, op=OP.add)
                rdt = smpool.tile([128, 1], F32)
                nc.vector.tensor_scalar(
                    rdt[:], rpre[:], cvec_t[:, 3:4], None, op0=OP.add
                )
                # bo[:, o] = rdt*pw[o] + pb[o]
                nc.vector.tensor_scalar(
                    bo[:], pwpb_t[:, 0::2], rdt[:], None, op0=OP.mult
                )
                nc.vector.tensor_add(bo[:], bo[:], pwpb_t[:, 1::2])

            def emit_out_h0():
                # all 32 channels for m-half 0 (quarters 0&1), ACT-heavy
                # since DVE is busy with deg; DMAs ride the scalar ring
                hs = slice(0, MH2)
                for og in range(OUT_CH // OG):
                    for g in range(OG):
                        o = og * OG + g
                        dst = otf[0][:, o * MH2 : (o + 1) * MH2]
                        if g < 3:
                            nc.vector.tensor_scalar(
                                dst, snew[:, hs],
                                pwpb_t[:, 2 * o : 2 * o + 1],
                                bo[:, o : o + 1], op0=OP.mult, op1=OP.add,
                            )
                        else:
                            nc.scalar.activation(
                                dst, snew[:, hs], ACTF.Identity,
                                bias=bo[:, o : o + 1],
                                scale=pwpb_t[:, 2 * o : 2 * o + 1],
                            )
                if og == OUT_CH // OG - 1:
                    for b in range(B_LOC):
                        osrc = otf[0][b * C : (b + 1) * C, :].rearrange(
                            "p (og o m) -> p og o m", og=4, m=MH2
                        )
                        nc.scalar.dma_start(
                            out_sh[b, 0].rearrange("og c o m -> c og o m"), osrc
                        )

            def emit_out_h1_part(qp):
                # quarter-slice (q = 2 + qp) of half 1 for all 32 channels
                qs2 = slice((2 + qp) * MBH, (3 + qp) * MBH)
                for og in range(OUT_CH // OG):
                    for g in range(OG):
                        o = og * OG + g
                        dst = otf[1][
                            :, o * MH2 + qp * MBH : o * MH2 + (qp + 1) * MBH
                        ]
                        if g < 5:
                            nc.vector.tensor_scalar(
                                dst, snew[:, qs2],
                                pwpb_t[:, 2 * o : 2 * o + 1],
                                bo[:, o : o + 1], op0=OP.mult, op1=OP.add,
                            )
                        else:
                            nc.scalar.activation(
                                dst, snew[:, qs2], ACTF.Identity,
                                bias=bo[:, o : o + 1],
                                scale=pwpb_t[:, 2 * o : 2 * o + 1],
                            )
                    if qp == 1 and og == OUT_CH // OG - 1:
                        # all channels done -> big DMAs on the (now idle)
                        # sync ring
                        for b in range(B_LOC):
                            osrc = otf[1][b * C : (b + 1) * C, :].rearrange(
                                "p (og o m) -> p og o m", og=4, m=MH2
                            )
                            nc.sync.dma_start(
                                out_sh[b, 1].rearrange("og c o m -> c og o m"),
                                osrc,
                            )

            # ---- schedule: x front-loaded, combine lags one quarter ----
            emit_x_quarter(0)
            nc.vector.tensor_scalar(
                ppr[:], pp_t[:], cvec_t[:, 2:3], None, op0=OP.mult
            )
            emit_x_quarter(1)
            emit_a_quarter(0)
            emit_x_quarter(2)
            emit_combine(0)
            emit_a_quarter(1)
            emit_x_quarter(3)
            emit_mlp_bo()
            emit_combine(1)
            emit_out_h0()
            emit_a_quarter(2)
            emit_combine(2)
            emit_out_h1_part(0)
            emit_a_quarter(3)
            emit_combine(3)
            emit_out_h1_part(1)

    nc.compile()
    return nc


def _get_bass():
    if "nc" not in _CACHE:
        _CACHE["nc"] = _build_bass()
    return _CACHE["nc"]


def _host_consts(kappa, alpha, w1, b1, w2, b2, pw, pb):
    kappa = float(np.asarray(kappa))
    alpha = float(np.asarray(alpha))
    w1 = np.asarray(w1, np.float32).reshape(16, 1)
    b1 = np.asarray(b1, np.float32).reshape(16)
    w2 = np.asarray(w2, np.float32).reshape(1, 16)
    b2 = np.asarray(b2, np.float32).reshape(1)
    pw = np.asarray(pw, np.float32).reshape(OUT_CH)
    pb = np.asarray(pb, np.float32).reshape(OUT_CH)

    kDT = DT * float(np.log1p(np.exp(kappa)))  # DT * softplus(kappa)

    # blockdiag "ones" carry the 1/F_DIM mean scale (exact in fp16)
    ones_bd = np.zeros((128, C), np.float16)
    for f in range(2):
        for c in range(C):
            ones_bd[f * C + c, c] = 1.0 / F_DIM

    w1r = np.tile(w1.T.astype(np.float32), (128, 1))  # [128,16]
    b1r = np.tile(b1[None, :], (128, 1)).astype(np.float32)
    w2r_dt = np.tile((DT * w2).astype(np.float32), (128, 1))  # [128,16]

    cvec = np.zeros((128, 4), np.float32)
    cvec[:, 0] = -kDT
    cvec[:, 1] = kDT
    cvec[:, 2] = DT * alpha
    # rdt = rpre + cvec3 where rpre = sum(w2r_dt * (elu+1));
    # true DT*r = sum(w2r_dt*elu) + DT*b2  ->  cvec3 = DT*b2 - sum(w2r_dt row)
    cvec[:, 3] = DT * b2[0] - float(w2r_dt[0].sum())

    pwpb = np.zeros((128, 2 * OUT_CH), np.float32)
    pwpb[:, 0::2] = pw[None, :]
    pwpb[:, 1::2] = pb[None, :]
    return ones_bd, w1r, b1r, w2r_dt, cvec, pwpb


def _host_prep(x, A, phys_prior, kappa, alpha, w1, b1, w2, b2, pw, pb):
    """Pack full inputs into per-core in_maps (fp16 blocked layouts)."""
    x = np.asarray(x, np.float32)
    A = np.asarray(A, np.float32)
    pp = np.ascontiguousarray(np.asarray(phys_prior, np.float32))

    # x_pe[b, q, f2*64+c, fp*128+mq] = x[b, 2*fp+f2, c, q*128+mq]
    x16 = x.astype(np.float16).reshape(B, NFP, 2, C, NQ, MBH)
    x_pe = np.ascontiguousarray(x16.transpose(0, 4, 2, 3, 1, 5)).reshape(
        B, NQ, 128, NFP * MBH
    )
    # a_pe[b, t, m0*64+c, m1*64+d] = A[b, t*32+2*m1+m0, c, d]
    A16 = A.astype(np.float16).reshape(B, NT, M_T // 2, 2, C, C)
    a_pe = np.ascontiguousarray(A16.transpose(0, 1, 3, 4, 2, 5)).reshape(
        B, NT, 128, (M_T // 2) * C
    )

    ones_bd, w1r, b1r, w2r_dt, cvec, pwpb = _host_consts(
        kappa, alpha, w1, b1, w2, b2, pw, pb
    )

    in_maps = []
    for i in range(N_CORES):
        sl = slice(i * B_LOC, (i + 1) * B_LOC)
        in_maps.append(
            {
                "x_sh": x_pe[sl],
                "a_sh": a_pe[sl],
                "pp_sh": pp[sl],
                "ones_bd": ones_bd,
                "w1r": w1r,
                "b1r": b1r,
                "w2r": w2r_dt,
                "cvec": cvec,
                "pwpb": pwpb,
            }
        )
    return in_maps


def _host_post(res):
    """Gather per-core fp16 [B_LOC,2,4,C,OG,MH2] outputs -> fp32 (B,O,C,M)."""
    out16 = np.concatenate(
        [res.results[i]["out"] for i in range(N_CORES)], axis=0
    )  # (B, 2, 4, C, OG, MH2)
    # o = og*OG + g, m = h*MH2 + mh
    out = out16.transpose(0, 2, 4, 3, 1, 5).reshape(B, OUT_CH, C, M)
    return np.ascontiguousarray(out).astype(np.float32)


def kernel(x, A, phys_prior, kappa, alpha, w1, b1, w2, b2, pw, pb):
    in_maps = _host_prep(
        x, A, phys_prior, kappa, alpha, w1, b1, w2, b2, pw, pb
    )
    nc = _get_bass()
    res = run_bass_kernel_spmd(nc, in_maps, list(range(N_CORES)))
    return _host_post(res)


if __name__ == "__main__":
    # smoke test with random data
    rng = np.random.default_rng(0)
    inputs = dict(
        x=rng.standard_normal((B, F_DIM, C, M)).astype(np.float32),
        A=rng.random((B, M, C, C)).astype(np.float32),
        phys_prior=rng.standard_normal((B, C, M)).astype(np.float32),
        kappa=np.float32(0.1),
        alpha=np.float32(0.05),
        w1=rng.standard_normal((16, 1)).astype(np.float32),
        b1=np.zeros(16, np.float32),
        w2=(rng.standard_normal((1, 16)) * 0.25).astype(np.float32),
        b2=np.zeros(1, np.float32),
        pw=rng.standard_normal(OUT_CH).astype(np.float32),
        pb=np.zeros(OUT_CH, np.float32),
    )
    out = kernel(**inputs)
    print("out", out.shape, out.dtype)
